# revision 2
# baseline (speedup 1.0000x reference)
"""AttentionMem Trainium2 Bass kernel (v3 — engine-balanced, bf16 datapath).

Problem: B=2, N=4096, M=1024, DIM=512, HEADS=8, DIM_HEAD=64.
  out = (softmax(LN(x)Wq @ concat(LN(x)Wk, mem)_peg^T / 8) @ concat(LN(x)Wv, mem)_peg) @ Wout + b_out

Sharding: core c = b*4 + g handles batch b and heads {2g, 2g+1} (128
channels). Each core computes a partial [4096, 512] output; host sums
the 4 group partials per batch and adds b_out.

Key structure (vs the fp32r baseline):
  - softmax scale 1/8 folded into wq on host; scores arrive pre-scaled.
  - exp is split between ScalarE (exact Exp) and a custom DVE op
    ANT_EXP16(x) = ((c0*x + c1)*x + c2)^16 (minimax fit of exp on
    [-3.5, 3.5], ~1.5e-3 rms on the real score distribution) so both
    engines exponentiate in parallel; exp output is bf16.
  - AV uses the transposed orientation: per key-chunk, lhsT = exp-scores
    [keys, 128-query slice] (bf16), rhs = v_rm [keys, 65] bf16 (64 dims
    + ones column) -> PSUM [128q, 65] accumulators; 65-row bf16 matmuls.
    Softmax denominator rides in column 64; normalization is a
    per-partition Copy-scale on ScalarE.
  - AV matmul emission runs two exp-tiles behind the score matmuls so
    the in-order PE queue never waits on the exp engines.
  - xn, projection weights, v, exp-scores, head outputs and wout are
    bf16 (rel-err budget 2e-2); stats/scores/PSUM accumulate in fp32.
  - v_rm and the output-head transposes go through dma_start_transpose
    (XBAR) on the DMA path instead of PE+copy.
  - PEG1D runs in column quarters split across DVE and GpSimd (Pool),
    emitted as soon as the gating k/v projection chunk is done.
  - attention chunk order is [1..38, 39, 0] so the PEG wrap (which
    touches chunks 0 and 39) is only needed late in each group.
"""

import os
import sys

import numpy as np

for _p in ("/opt/trn_rl_repo", "/root/.axon_site/_ro/trn_rl_repo"):
    if os.path.isdir(_p) and _p not in sys.path:
        sys.path.insert(0, _p)

from contextlib import ExitStack

import concourse.bacc as bacc
import concourse.bass as bass
import concourse.mybir as mybir
import concourse.tile as tile
from concourse.bass_utils import run_bass_kernel_spmd

AF = mybir.ActivationFunctionType
ALU = mybir.AluOpType
F32 = mybir.dt.float32
F32R = mybir.dt.float32r
BF16 = mybir.dt.bfloat16

B = 2
N = 4096
M = 1024
DIM = 512
NK = N + M  # 5120
HSL = 128  # head-slice channels per core (2 heads x 64)
DH = 64
EPS = 1e-5
L = NK - 1  # PEG length (positions 1..NK-1)
LP = L + 1  # padded to multiple of 32 -> 5120
NCH = NK // 128  # 40 k chunks of 128
QG = 512  # q group size
NQG = N // QG  # 8
SCALE = DH ** -0.5

# quad^16 exp fit on [-3.5, 3.5] (scaled-score domain)
EC0 = 0.00194353
EC1 = 0.06279615
EC2 = 1.00005614


# ---- custom DVE op: ANT_EXP16(x) = ((C0 x + C1) x + C2)^16 ------------------
def _register_exp16():
    import concourse.dve_ops as dve_ops
    from concourse.dve_spec import C0, C1, C2, Spec, Src0, lower, sq
    from concourse.dve_uop import DveOpSpec

    name = "ANT_EXP16"
    for o in dve_ops.OPS:
        if o.name == name:
            return o
    body = sq(sq(sq(sq((C0 * Src0 + C1) * Src0 + C2))))
    spec = Spec(
        body=body,
        reference=lambda in0, in1, s0, s1, imm2: ((s0 * in0 + s1) * in0 + imm2)
        ** 16,
    )
    row = dve_ops._CUSTOM_DVE_ROW_BASE + len(dve_ops.OPS)
    assert row < 0x20
    dve_ops._SUB_OPCODE_FOR_NAME[name] = row
    shas = {}
    for ver in ("v3", "v4"):
        tmp = DveOpSpec(name=name, opcode=row, uops=lower(spec, ver=ver),
                        rd1_en=False)
        shas[ver] = tmp.sha(ver)
    op = dve_ops.DveOp(name, spec, subdim=False, uops_sha=shas)
    dve_ops.OPS.append(op)
    dve_ops.CUSTOM_DVE_SPECS[name] = spec
    return op


EXP16 = _register_exp16()


def r(ap):
    return ap.bitcast(F32R)


# ---- PEG1D pieces (generalized column blocks) -------------------------------
def _peg_core(eng, P, O, w0, w1, w2, b2, lo, hi):
    """O[lo:hi] = P + w1*P + 2b with p1 chunk taps."""
    eng.tensor_scalar(out=r(O[:, lo:hi]), in0=P[:, lo:hi], scalar1=w1,
                      scalar2=b2, op0=ALU.mult, op1=ALU.add)
    eng.tensor_tensor(out=r(O[:, lo:hi]), in0=P[:, lo:hi],
                      in1=O[:, lo:hi], op=ALU.add)
    P3 = P[:, lo:hi].rearrange("p (c t) -> p c t", t=32)
    O3 = O[:, lo:hi].rearrange("p (c t) -> p c t", t=32)
    eng.scalar_tensor_tensor(out=r(O3[:, :, 1:32]), in0=P3[:, :, 0:31],
                             scalar=w0, in1=O3[:, :, 1:32],
                             op0=ALU.mult, op1=ALU.add)
    eng.scalar_tensor_tensor(out=r(O3[:, :, 0:31]), in0=P3[:, :, 1:32],
                             scalar=w2, in1=O3[:, :, 0:31],
                             op0=ALU.mult, op1=ALU.add)


def _peg_p2_taps(eng, P, O, w0, w2, c0, c1):
    """p2 neighbor taps for full shifted chunks c0..c1-1 (chunk m covers
    i in [16+32m, 48+32m))."""
    if c1 <= c0:
        return
    lo = 16 + 32 * c0
    n = c1 - c0
    PB = P[:, lo:lo + n * 32].rearrange("p (c t) -> p c t", t=32)
    OB = O[:, lo:lo + n * 32].rearrange("p (c t) -> p c t", t=32)
    eng.scalar_tensor_tensor(out=r(OB[:, :, 1:32]), in0=PB[:, :, 0:31],
                             scalar=w0, in1=OB[:, :, 1:32],
                             op0=ALU.mult, op1=ALU.add)
    eng.scalar_tensor_tensor(out=r(OB[:, :, 0:31]), in0=PB[:, :, 1:32],
                             scalar=w2, in1=OB[:, :, 0:31],
                             op0=ALU.mult, op1=ALU.add)


def emit_peg_block(eng, src, dst, w3, b2, b0, b1):
    """PEG1D main work for output region [b0, b1) (cols of the L-length
    region), b0/b1 multiples of 32, plus cls col when b0 == 0. The wrap
    chunk is in emit_peg_wrap."""
    w0, w1, w2 = w3[:, 0:1], w3[:, 1:2], w3[:, 2:3]
    P = src[:, 1:1 + LP]
    O = dst[:, 1:1 + LP]
    if b0 == 0:
        eng.tensor_copy(r(dst[:, 0:1]), src[:, 0:1])
    _peg_core(eng, P, O, w0, w1, w2, b2, b0, b1)
    lo2, hi2 = max(16, b0), min(b1, L - 16)
    if hi2 > lo2:
        eng.scalar_tensor_tensor(out=r(O[:, lo2:hi2]), in0=P[:, lo2:hi2],
                                 scalar=w1, in1=O[:, lo2:hi2],
                                 op0=ALU.mult, op1=ALU.add)
    m0 = 0 if b0 == 0 else (b0 - 16) // 32
    m1 = 158 if b1 >= LP else (b1 - 16) // 32
    _peg_p2_taps(eng, P, O, w0, w2, m0, m1)
    if b1 >= LP:
        # trimmed chunk 158: i=5103 is wrap, i=5102 right tap = 0
        c158 = 16 + 158 * 32  # 5072
        eng.scalar_tensor_tensor(out=r(O[:, c158 + 1:c158 + 31]),
                                 in0=P[:, c158:c158 + 30], scalar=w0,
                                 in1=O[:, c158 + 1:c158 + 31],
                                 op0=ALU.mult, op1=ALU.add)
        eng.scalar_tensor_tensor(out=r(O[:, c158:c158 + 30]),
                                 in0=P[:, c158 + 1:c158 + 31], scalar=w2,
                                 in1=O[:, c158:c158 + 30],
                                 op0=ALU.mult, op1=ALU.add)


def emit_peg_wrap(eng, src, dst, w3, b2, wbuf, wtmp):
    """Wrap chunk of the shifted grid: sx[0:32] = [P[L-16:L], P[0:16]].
    Requires the [0:...) and (..., LP) main blocks to be complete."""
    w0, w1, w2 = w3[:, 0:1], w3[:, 1:2], w3[:, 2:3]
    P = src[:, 1:1 + LP]
    O = dst[:, 1:1 + LP]
    eng.tensor_copy(wbuf[:, 0:16], P[:, L - 16:L])
    eng.tensor_copy(wbuf[:, 16:32], P[:, 0:16])
    eng.tensor_scalar_mul(wtmp[:], wbuf[:], w1)
    eng.scalar_tensor_tensor(out=wtmp[:, 1:32], in0=wbuf[:, 0:31],
                             scalar=w0, in1=wtmp[:, 1:32],
                             op0=ALU.mult, op1=ALU.add)
    eng.scalar_tensor_tensor(out=wtmp[:, 0:31], in0=wbuf[:, 1:32],
                             scalar=w2, in1=wtmp[:, 0:31],
                             op0=ALU.mult, op1=ALU.add)
    eng.tensor_tensor(out=r(O[:, L - 16:L]), in0=wtmp[:, 0:16],
                      in1=O[:, L - 16:L], op=ALU.add)
    eng.tensor_tensor(out=r(O[:, 0:16]), in0=wtmp[:, 16:32],
                      in1=O[:, 0:16], op=ALU.add)


def build_module():
    nc = bacc.Bacc("TRN2", target_bir_lowering=False, debug=False,
                   num_devices=8)

    d_x = nc.dram_tensor("x", [N, DIM], F32, kind="ExternalInput")
    d_mem = nc.dram_tensor("memslice", [M, HSL], F32R, kind="ExternalInput")
    d_wq = nc.dram_tensor("wq", [DIM, HSL], F32, kind="ExternalInput")
    d_wk = nc.dram_tensor("wk", [DIM, HSL], F32, kind="ExternalInput")
    d_wv = nc.dram_tensor("wv", [DIM, HSL], F32, kind="ExternalInput")
    d_bq = nc.dram_tensor("bq", [HSL, 1], F32, kind="ExternalInput")
    d_bk = nc.dram_tensor("bk", [HSL, 1], F32, kind="ExternalInput")
    d_bv = nc.dram_tensor("bv", [HSL, 1], F32, kind="ExternalInput")
    d_wout = nc.dram_tensor("wout", [HSL, DIM], F32, kind="ExternalInput")
    d_pkw = nc.dram_tensor("pkw", [HSL, 3], F32, kind="ExternalInput")
    d_pkb2 = nc.dram_tensor("pkb2", [HSL, 1], F32, kind="ExternalInput")
    d_pvw = nc.dram_tensor("pvw", [HSL, 3], F32, kind="ExternalInput")
    d_pvb2 = nc.dram_tensor("pvb2", [HSL, 1], F32, kind="ExternalInput")
    d_ident = nc.dram_tensor("ident", [128, 128], F32R, kind="ExternalInput")
    d_ones = nc.dram_tensor("ones", [128, 128], F32, kind="ExternalInput")
    d_out = nc.dram_tensor("out", [N, DIM], F32, kind="ExternalOutput")

    with tile.TileContext(nc) as tc, ExitStack() as ctx:
        cpool = ctx.enter_context(tc.tile_pool(name="consts", bufs=1))
        ppool = ctx.enter_context(tc.tile_pool(name="persist", bufs=1))
        pegpool = ctx.enter_context(tc.tile_pool(name="peg", bufs=1))
        xpool = ctx.enter_context(tc.tile_pool(name="xtiles", bufs=3))
        stpool = ctx.enter_context(tc.tile_pool(name="stats", bufs=4))
        xnpool = ctx.enter_context(tc.tile_pool(name="xn", bufs=2))
        xntpool = ctx.enter_context(tc.tile_pool(name="xnt", bufs=1))
        espool = ctx.enter_context(tc.tile_pool(name="exps", bufs=5))
        finpool = ctx.enter_context(tc.tile_pool(name="fin", bufs=2))
        sgpool = ctx.enter_context(tc.tile_pool(name="stage", bufs=2))
        vstpool = ctx.enter_context(tc.tile_pool(name="vstage", bufs=3))
        ps = ctx.enter_context(tc.tile_pool(name="ps", bufs=1, space="PSUM"))

        # ---- constants / weights ----
        def cload(name, dram, shape, chunked=False, dt=F32):
            t = cpool.tile(shape, dt, tag=name, name=name)
            src = dram.ap()
            dst = t[:]
            if chunked:
                src = src.rearrange("(c p) o -> p c o", p=128)
                dst = dst.rearrange("p (c o) -> p c o", o=128)
            nc.sync.dma_start(dst, src)
            return t

        wq = cload("wq", d_wq, [128, 512], chunked=True)
        wk = cload("wk", d_wk, [128, 512], chunked=True)
        wv = cload("wv", d_wv, [128, 512], chunked=True)
        bq = cload("bq", d_bq, [128, 1])
        bk = cload("bk", d_bk, [128, 1])
        bv = cload("bv", d_bv, [128, 1])
        wout = cload("wout", d_wout, [128, 512])
        pkw = cload("pkw", d_pkw, [128, 3])
        pkb2 = cload("pkb2", d_pkb2, [128, 1])
        pvw = cload("pvw", d_pvw, [128, 3])
        pvb2 = cload("pvb2", d_pvb2, [128, 1])
        ident = cload("ident", d_ident, [128, 128], dt=F32R)
        ones = cload("ones", d_ones, [128, 128])

        # bf16 copies of the matmul weights (device-side convert, one-time)
        wq16 = cpool.tile([128, 512], BF16, tag="wq16", name="wq16")
        wk16 = cpool.tile([128, 512], BF16, tag="wk16", name="wk16")
        wv16 = cpool.tile([128, 512], BF16, tag="wv16", name="wv16")
        wout16 = cpool.tile([128, 512], BF16, tag="wout16", name="wout16")
        ident16 = cpool.tile([128, 128], BF16, tag="ident16", name="ident16")
        for dst16, src32 in ((wq16, wq), (wk16, wk), (wv16, wv),
                             (wout16, wout)):
            nc.gpsimd.tensor_copy(dst16[:], src32[:])
        nc.gpsimd.tensor_copy(ident16[:], ident[:].bitcast(F32))

        # ---- persistent tensors ----
        qT = ppool.tile([128, N], F32, tag="qT", name="qT")
        kT_raw = pegpool.tile([128, NK + 1], F32, tag="kT_raw", name="kT_raw")
        kT = ppool.tile([128, NK + 1], F32, tag="kT", name="kT")
        vT_raw = pegpool.tile([128, NK + 1], F32, tag="vT_raw", name="vT_raw")
        vT = pegpool.tile([128, NK + 1], F32, tag="vT", name="vT")
        vT16 = pegpool.tile([128, LP], BF16, tag="vT16", name="vT16")
        v_rm = ppool.tile([128, NCH * 130], BF16, tag="v_rm", name="v_rm")
        wbufk = pegpool.tile([128, 32], F32, tag="wbufk", name="wbufk")
        wtmpk = pegpool.tile([128, 32], F32, tag="wtmpk", name="wtmpk")
        wbufv = pegpool.tile([128, 32], F32, tag="wbufv", name="wbufv")
        wtmpv = pegpool.tile([128, 32], F32, tag="wtmpv", name="wtmpv")

        nc.vector.memset(kT_raw[:, NK:NK + 1], 0.0)
        nc.vector.memset(vT_raw[:, NK:NK + 1], 0.0)

        # v_rm ones columns at 64 / 129 per chunk (written once; the
        # dma transposes only touch [:, :, 0:64] of each 65-group)
        vr3 = v_rm.rearrange("p (c t) -> p c t", t=130)
        nc.scalar.copy(vr3[:, :, 64:65].rearrange("p c t -> p (c t)"),
                       ones[:, 0:NCH])
        nc.scalar.copy(vr3[:, :, 129:130].rearrange("p c t -> p (c t)"),
                       ones[:, 0:NCH])

        # ---- phase A: mem -> kT_raw/vT_raw cols N..NK (PE warmup) ----
        mp = ps.tile([128, 1024], F32, tag="sc", bufs=3, name="memtr")
        for m in range(8):
            mt = xpool.tile([128, 128], F32R, tag="memt", name=f"memt_{m}")
            nc.sync.dma_start(mt[:], d_mem.ap()[m * 128:(m + 1) * 128, :])
            sl = mp[:, (m % 8) * 128:(m % 8) * 128 + 128]
            nc.tensor.transpose(r(sl), mt[:], ident[:])
            nc.scalar.copy(kT_raw[:, N + m * 128:N + (m + 1) * 128], sl)
        nc.scalar.copy(vT_raw[:, N:NK], kT_raw[:, N:NK])

        # ---- phase B: LN + transpose + projections, per seq group ----
        # PEG quarter (b0, b1) gated on the last seq group writing
        # positions < b1 + 17 for both k and v.
        QTR = [(640 * i, 640 * (i + 1)) for i in range(8)]
        peg_plan = {
            1: [("k", 0, nc.vector), ("v", 0, nc.vector)],
            2: [("k", 1, nc.vector), ("v", 1, nc.vector)],
            3: [("k", 2, nc.vector), ("v", 2, nc.vector)],
            5: [("k", 3, nc.vector), ("v", 3, nc.vector)],
            6: [("k", 4, nc.vector), ("v", 4, nc.vector)],
            7: [("k", 5, nc.vector), ("v", 5, nc.vector),
                ("k", 6, nc.vector), ("v", 6, nc.vector),
                ("k", 7, nc.vector), ("v", 7, nc.vector)],
        }
        peg_args = {
            "k": (kT_raw, kT, pkw, pkb2, wbufk, wtmpk),
            "v": (vT_raw, vT, pvw, pvb2, wbufv, wtmpv),
        }
        vrm_hw = [0]  # v_rm chunk high-water mark

        def vrm_transposes(chunks):
            # XBAR transpose needs a contiguous 2D output; stage then do a
            # strided copy into the 65-col groups on GpSimd.
            for c in chunks:
                stg = vstpool.tile([128, 128], BF16, tag="vstg", bufs=3,
                                   name=f"vstg_{c}")
                nc.sync.dma_start_transpose(
                    stg[:], vT16[:, c * 128:(c + 1) * 128])
                dst = v_rm[:, c * 130:(c + 1) * 130]
                dst3 = dst.rearrange("p (g t) -> p g t", t=65)[:, :, 0:64]
                nc.gpsimd.tensor_copy(
                    dst3, stg.rearrange("p (g t) -> p g t", t=64))

        def vt16_convert(qi):
            b0, b1 = QTR[qi]
            lo = max(0, b0 - 16)
            nc.gpsimd.tensor_copy(vT16[:, lo:b1 - 16], vT[:, lo:b1 - 16])

        for sg in range(NQG):
            xnT = xntpool.tile([128, 2048], BF16, tag="xnT", bufs=2,
                               name=f"xnT_{sg}")
            xt = xpool.tile([128, 2048], F32, tag="xt", bufs=4,
                            name=f"xt_{sg}")
            xt4 = xt.rearrange("p (g c) -> p g c", c=512)
            nc.sync.dma_start(
                xt4, d_x.ap()[sg * 512:(sg + 1) * 512, :]
                .rearrange("(g p) c -> p g c", p=128))
            # stats: sum on DVE; sum-of-squares via Square+accum on ScalarE
            s4 = stpool.tile([128, 4], F32, tag="s4", name=f"s4_{sg}")
            nc.vector.tensor_reduce(s4[:], xt4, mybir.AxisListType.X,
                                    op=ALU.add)
            ss4 = stpool.tile([128, 4], F32, tag="ss4", name=f"ss4_{sg}")
            sqs = xpool.tile([128, 512], F32, tag="sqs", bufs=1,
                             name=f"sqs_{sg}")
            for g in range(4):
                nc.scalar.activation(sqs[:], xt4[:, g, :], AF.Square,
                                     accum_out=ss4[:, g:g + 1])
            mean = stpool.tile([128, 4], F32, tag="mean", name=f"mean_{sg}")
            nc.vector.tensor_scalar(out=mean[:], in0=s4[:],
                                    scalar1=1.0 / DIM, scalar2=None,
                                    op0=ALU.mult)
            m2 = stpool.tile([128, 4], F32, tag="m2", name=f"m2_{sg}")
            nc.vector.tensor_tensor(out=m2[:], in0=mean[:], in1=mean[:],
                                    op=ALU.mult)
            var = stpool.tile([128, 4], F32, tag="var", name=f"var_{sg}")
            nc.vector.scalar_tensor_tensor(out=var[:], in0=ss4[:],
                                           scalar=1.0 / DIM, in1=m2[:],
                                           op0=ALU.mult, op1=ALU.subtract)
            t1 = stpool.tile([128, 4], F32, tag="t1", name=f"t1_{sg}")
            nc.vector.tensor_scalar(out=t1[:], in0=var[:], scalar1=EPS,
                                    scalar2=None, op0=ALU.add)
            r1 = stpool.tile([128, 4], F32, tag="r1", name=f"r1_{sg}")
            nc.vector.reciprocal(r1[:], t1[:])
            rstd = stpool.tile([128, 4], F32, tag="rstd", name=f"rstd_{sg}")
            nc.scalar.activation(rstd[:], r1[:], AF.Sqrt)
            nmrstd = stpool.tile([128, 4], F32, tag="nmrstd",
                                 name=f"nmrstd_{sg}")
            nc.vector.scalar_tensor_tensor(out=nmrstd[:], in0=mean[:],
                                           scalar=-1.0, in1=rstd[:],
                                           op0=ALU.mult, op1=ALU.mult)
            # xn (bf16) on Pool, PE-transpose (bf16) into one PSUM slot,
            # then a single wide 2-byte copy into xnT
            pt = ps.tile([128, 2048], BF16, tag="sc", bufs=3,
                         name=f"pt_{sg}")
            for g in range(4):
                xng = xnpool.tile([128, 512], BF16, tag="xn", bufs=3,
                                  name=f"xng_{sg}_{g}")
                if g % 2 == 0:
                    nc.vector.tensor_scalar(out=xng[:], in0=xt4[:, g, :],
                                            scalar1=rstd[:, g:g + 1],
                                            scalar2=nmrstd[:, g:g + 1],
                                            op0=ALU.mult, op1=ALU.add)
                else:
                    nc.scalar.activation(xng[:], xt4[:, g, :], AF.Identity,
                                         bias=nmrstd[:, g:g + 1],
                                         scale=rstd[:, g:g + 1])
                for c in range(4):
                    nc.tensor.transpose(
                        pt[:, c * 512 + g * 128:c * 512 + (g + 1) * 128],
                        xng[:, c * 128:(c + 1) * 128], ident16[:])
            if sg % 2 == 0:
                nc.vector.tensor_copy(xnT[:], pt[:])
            else:
                nc.scalar.copy(xnT[:], pt[:])
            for i, (w_sb, bias, dst, deng) in enumerate(
                    ((wk16, bk, kT_raw, nc.vector),
                     (wv16, bv, vT_raw, nc.scalar),
                     (wq16, bq, qT, nc.vector))):
                pp = ps.tile([128, 512], F32, tag="sc", bufs=3,
                             name=f"pp_{sg}_{i}")
                for c in range(4):
                    nc.tensor.matmul(pp[:], w_sb[:, c * 128:(c + 1) * 128],
                                     xnT[:, c * 512:(c + 1) * 512],
                                     start=(c == 0), stop=(c == 3))
                if deng is nc.scalar:
                    nc.scalar.activation(dst[:, sg * 512:(sg + 1) * 512],
                                         pp[:], AF.Identity, bias=bias[:, 0:1])
                else:
                    deng.tensor_scalar_add(
                        r(dst[:, sg * 512:(sg + 1) * 512]), pp[:], bias[:])
            for name_, qi, eng in peg_plan.get(sg, ()):
                src, dst, w3, b2, _, _ = peg_args[name_]
                b0, b1 = QTR[qi]
                emit_peg_block(eng, src[:], dst[:], w3[:], b2[:], b0, b1)
                if name_ == "v":
                    vt16_convert(qi)
                    cmax = (b1 - 16) // 128  # chunks with (c+1)*128 <= b1-16
                    vrm_transposes(
                        [c for c in range(vrm_hw[0], cmax)
                         if c not in (0, 39)])
                    vrm_hw[0] = cmax

        emit_peg_wrap(nc.vector, kT_raw[:], kT[:], pkw[:], pkb2[:],
                      wbufk, wtmpk)
        emit_peg_wrap(nc.vector, vT_raw[:], vT[:], pvw[:], pvb2[:],
                      wbufv, wtmpv)
        # wrap-touched v cols -> vT16 -> v_rm chunks 39 and 0
        nc.gpsimd.tensor_copy(vT16[:, 0:32], vT[:, 0:32])
        nc.gpsimd.tensor_copy(vT16[:, 5088:LP], vT[:, 5088:LP])
        vrm_transposes([c for c in range(vrm_hw[0], 39)] + [39, 0])

        # ---- phase D: attention ----
        # chunk order [1..38, 39, 0]: wrap-dependent chunks last.
        SEQ = list(range(1, 39)) + [39, 0]

        def dve_exp(qg, h, pj):
            if qg == 0 and h == 0:
                return pj in (13, 15, 17)
            return (pj % 2) == 1 and pj < 19

        pending_fin = []  # deferred finalize closures
        pending_av = []  # software-pipelined AV emission (depth 2)
        oh_tiles = {}  # qg -> [128, 512] bf16 stacked-head output tile

        def stage1(qg, h, otp):
            """normalize otp -> ohBoth halves; h1 also emits the XBAR
            transposes into trs."""
            def go():
                dn = stpool.tile([128, 4], F32, tag="dn", bufs=2,
                                 name=f"dn_{qg}_{h}")
                den = otp[:, 0:260].rearrange(
                    "p (q t) -> p q t", t=65)[:, :, 64:65]
                nc.vector.reciprocal_approx_fast(
                    dn[:], den.rearrange("p q t -> p (q t)"))
                if qg not in oh_tiles:
                    oh_tiles[qg] = finpool.tile([128, 512], BF16, tag="oh",
                                                bufs=2, name=f"oh_{qg}")
                oh = oh_tiles[qg]
                for qs in range(4):
                    nc.scalar.activation(
                        oh[:, qs * 128 + 64 * h:qs * 128 + 64 * h + 64],
                        otp[:, qs * 65:qs * 65 + 64],
                        AF.Copy, scale=dn[:, qs:qs + 1])
                if h == 1:
                    trs = finpool.tile([128, 512], BF16, tag="trs", bufs=2,
                                       name=f"trs_{qg}")
                    oh_tiles[f"t{qg}"] = trs
                    for qs in range(4):
                        nc.sync.dma_start_transpose(
                            trs[:, qs * 128:(qs + 1) * 128],
                            oh[:, qs * 128:(qs + 1) * 128])
            return go

        def stage2(qg, qs):
            """trs -> out-projection -> DMA for one 128-query block."""
            def go():
                trs = oh_tiles[f"t{qg}"]
                fp = ps.tile([128, 512], F32, tag="otp1", bufs=1,
                             name=f"fpo_{qg}_{qs}")
                nc.tensor.matmul(fp[:], trs[:, qs * 128:(qs + 1) * 128],
                                 wout16[:], start=True, stop=True)
                st = sgpool.tile([128, 512], F32, tag="ost", bufs=2,
                                 name=f"st_{qg}_{qs}")
                nc.scalar.copy(st[:], fp[:])
                nc.sync.dma_start(
                    d_out.ap()[qg * 512 + qs * 128:
                               qg * 512 + (qs + 1) * 128, :], st[:])
                if qs == 3:
                    oh_tiles.pop(qg)
                    oh_tiles.pop(f"t{qg}")
            return go

        otp_tiles = {}

        def emit_av(job):
            # PSUM has_written bits are per-bank and reset on any start=True
            # to that bank, so interleaved groups must use start=True only on
            # the very first matmul into the tile; unset bits make each
            # group's first write an overwrite.
            es, qg, h, pis = job
            otp = otp_tiles[(qg, h)]
            for ci, pi in enumerate(pis):
                c = SEQ[pi]
                vsl = v_rm[:, c * 130 + 65 * h:c * 130 + 65 * h + 65]
                for qs in range(4):
                    nc.tensor.matmul(
                        otp[:, qs * 65:qs * 65 + 65],
                        es[:, ci * 512 + qs * 128:ci * 512 + (qs + 1) * 128],
                        vsl, start=(pi == 0 and qs == 0), stop=(pi == 39),
                        skip_group_check=True)

        for qg in range(NQG):
            q0 = qg * QG
            for h in range(2):
                hp = 64 * h
                shape = [128, 260] if h == 0 else [128, 512]
                otp_tiles[(qg, h)] = ps.tile(shape, F32, tag=f"otp{h}",
                                             bufs=1, name=f"otp_{qg}_{h}")
                for pj in range(20):
                    pis = (2 * pj, 2 * pj + 1)
                    if pj in (2, 4, 6, 8, 10) and pending_fin:
                        sched = [f for p, f in pending_fin if p == pj]
                        pending_fin[:] = [(p, f) for p, f in pending_fin
                                          if p != pj]
                        for fn in sched:
                            fn()
                    sc = ps.tile([128, 1024], F32, tag="sc", bufs=3,
                                 name=f"sc_{qg}_{h}_{pj}")
                    for ci, pi in enumerate(pis):
                        c = SEQ[pi]
                        nc.tensor.matmul(
                            sc[:, ci * 512:(ci + 1) * 512],
                            r(kT[hp:hp + 64, c * 128:(c + 1) * 128]),
                            r(qT[hp:hp + 64, q0:q0 + QG]),
                            start=True, stop=True)
                    es = espool.tile([128, 1024], BF16, tag="es", bufs=5,
                                     name=f"es_{qg}_{h}_{pj}")
                    if dve_exp(qg, h, pj):
                        nc.vector._custom_dve(EXP16, out=es[:], in0=sc[:],
                                              s0=EC0, s1=EC1, imm2=EC2)
                    else:
                        nc.scalar.activation(es[:], sc[:], AF.Exp)
                    pending_av.append((es, qg, h, pis))
                    if len(pending_av) > 2:
                        emit_av(pending_av.pop(0))
                pending_fin.append((2, stage1(qg, h, otp_tiles[(qg, h)])))
                if h == 1:
                    for qs in range(4):
                        pending_fin.append((4 + 2 * qs, stage2(qg, qs)))
        while pending_av:
            emit_av(pending_av.pop(0))
        for _, fn in sorted(pending_fin, key=lambda t: t[0]):
            fn()
        pending_fin.clear()

    nc.compile()
    return nc


_NC = None


def _get_nc():
    global _NC
    if _NC is None:
        _NC = build_module()
    return _NC


def make_in_maps(x, mem, ln_g, ln_b, w_qkv, w_out, b_out, pk_w, pk_b,
                 pv_w, pv_b):
    x = np.asarray(x, np.float32)
    mem = np.asarray(mem, np.float32)
    ln_g = np.asarray(ln_g, np.float32)
    ln_b = np.asarray(ln_b, np.float32)
    w_qkv = np.asarray(w_qkv, np.float32)
    w_out = np.asarray(w_out, np.float32)
    pk_w = np.asarray(pk_w, np.float32)
    pk_b = np.asarray(pk_b, np.float32)
    pv_w = np.asarray(pv_w, np.float32)
    pv_b = np.asarray(pv_b, np.float32)

    wqkv_g = w_qkv * ln_g[:, None]
    bias_row = ln_b @ w_qkv  # [3*INNER]
    ident = np.eye(128, dtype=np.float32)
    onesm = np.ones((128, 128), dtype=np.float32)
    in_maps = []
    for core in range(8):
        b, g = divmod(core, 4)
        hs = slice(128 * g, 128 * (g + 1))
        in_maps.append({
            "x": np.ascontiguousarray(x[b]),
            "memslice": np.ascontiguousarray(mem[b][:, hs]),
            # softmax scale folded into the q projection
            "wq": np.ascontiguousarray(
                SCALE * wqkv_g[:, 128 * g:128 * (g + 1)]),
            "wk": np.ascontiguousarray(
                wqkv_g[:, 512 + 128 * g:512 + 128 * (g + 1)]),
            "wv": np.ascontiguousarray(
                wqkv_g[:, 1024 + 128 * g:1024 + 128 * (g + 1)]),
            "bq": np.ascontiguousarray(
                SCALE * bias_row[128 * g:128 * (g + 1)].reshape(128, 1)),
            "bk": np.ascontiguousarray(
                bias_row[512 + 128 * g:512 + 128 * (g + 1)].reshape(128, 1)),
            "bv": np.ascontiguousarray(
                bias_row[1024 + 128 * g:1024 + 128 * (g + 1)].reshape(128, 1)),
            "wout": np.ascontiguousarray(w_out[hs, :]),
            "pkw": np.ascontiguousarray(pk_w[hs, 0, :]),
            "pkb2": np.ascontiguousarray(2.0 * pk_b[hs].reshape(128, 1)),
            "pvw": np.ascontiguousarray(pv_w[hs, 0, :]),
            "pvb2": np.ascontiguousarray(2.0 * pv_b[hs].reshape(128, 1)),
            "ident": ident,
            "ones": onesm,
        })
    return in_maps


def kernel(x, mem, ln_g, ln_b, w_qkv, w_out, b_out, pk_w, pk_b, pv_w, pv_b):
    nc = _get_nc()
    in_maps = make_in_maps(x, mem, ln_g, ln_b, w_qkv, w_out, b_out, pk_w,
                           pk_b, pv_w, pv_b)
    res = run_bass_kernel_spmd(nc, in_maps, list(range(8))).results
    b_out = np.asarray(b_out, np.float32)
    out = np.empty((B, N, DIM), np.float32)
    for b in range(B):
        acc = res[4 * b]["out"].astype(np.float32).copy()
        for g in range(1, 4):
            acc += res[4 * b + g]["out"]
        out[b] = acc + b_out[None, :]
    return out
